# revision 1
# baseline (speedup 1.0000x reference)
"""Trainium2 Bass kernel for nn_BiLSTMGenerator (BiLSTM -> LSTM -> dense).

Data-parallel over batch on 8 cores; per core B_shard = 2048 = 16 b-tiles.
Elementwise state is batch-major [128 batch partitions, tiles*features];
matmul activations are feature-major via per-step PE transposes of h.

Phase 1 interleaves the forward and backward H=16 LSTMs (two independent
chains, each split into 2 batch-half sub-chains), storing transposed h of
both directions to DRAM scratch hfb[t] = [hfT(16); hbT(16)] in bf16.

Phase 2 runs the mid H=64 LSTM + fused dense tap, 4 independent 4-tile
group chains per step. lhsT buffer V2 (double-buffered, alternating per
step): rows 0:32 hfbT (DMA), 32:64 ones+zeros (constant), 64:128 hmT
(PE-transpose evac). Mid matmul rhs is [128, 257]; column 256 taps
Wd @ h_m[t-1] + bd so the dense layer rides the same matmul.

Gate order is (i, f, o, g) so sigmoid spans a contiguous column range.
Biases ride the ones row. All matmul I/O bf16, PSUM fp32, cell states
bf16 (offline-validated: ~4e-3 rel err vs fp32 reference).
Two of four tensor-products per chain run on GPSIMD to unload DVE.
"""
import sys

sys.path.insert(0, "/opt/trn_rl_repo")

import numpy as np
import ml_dtypes

BF16NP = ml_dtypes.bfloat16

T, B, IN, H1, H2 = 216, 16384, 8, 16, 64
NCORES = 8
BS = B // NCORES  # 2048
NT = BS // 128  # 16 b-tiles


def _perm4(H):
    # torch gate order (i, f, g, o) -> (i, f, o, g)
    return np.concatenate(
        [np.arange(0, 2 * H), np.arange(3 * H, 4 * H), np.arange(2 * H, 3 * H)]
    )


def build_program(t_steps=T):
    import concourse.bass as bass
    import concourse.tile as tile
    from concourse import bacc, mybir
    from contextlib import ExitStack

    F32 = mybir.dt.float32
    BF = mybir.dt.bfloat16
    AF = mybir.ActivationFunctionType

    nc = bacc.Bacc("TRN2", target_bir_lowering=False, debug=False)

    xpad_d = nc.declare_dram_parameter("xpad", [t_steps, 16, BS], BF, isOutput=False)
    rhsf_d = nc.declare_dram_parameter("rhs_f", [25, 64], BF, isOutput=False)
    rhsb_d = nc.declare_dram_parameter("rhs_b", [25, 64], BF, isOutput=False)
    rhsm_d = nc.declare_dram_parameter("rhs_m", [128, 257], BF, isOutput=False)
    rhsd_d = nc.declare_dram_parameter("rhs_d", [64, 1], BF, isOutput=False)
    h0f_d = nc.declare_dram_parameter("h0fT", [16, BS], BF, isOutput=False)
    h0b_d = nc.declare_dram_parameter("h0bT", [16, BS], BF, isOutput=False)
    h0m_d = nc.declare_dram_parameter("h0mT", [64, BS], BF, isOutput=False)
    c0_d = nc.declare_dram_parameter("c0", [128, 96 * NT], BF, isOutput=False)
    id_d = nc.declare_dram_parameter("ident", [128, 128], BF, isOutput=False)
    ones_d = nc.declare_dram_parameter("onespad", [32, BS], BF, isOutput=False)
    out_d = nc.declare_dram_parameter("out", [128, t_steps * NT], F32, isOutput=True)
    hfb_d = nc.dram_tensor("hfb_scratch", [t_steps, 32, BS], BF)

    with tile.TileContext(nc) as tc, ExitStack() as ctx:
        const = ctx.enter_context(tc.tile_pool(name="const", bufs=1))
        state = ctx.enter_context(tc.tile_pool(name="state", bufs=1))
        work = ctx.enter_context(tc.tile_pool(name="work", bufs=2))

        # constants
        rhsf = const.tile([25, 64], BF, tag="rhsf")
        rhsb = const.tile([25, 64], BF, tag="rhsb")
        rhsm = const.tile([128, 257], BF, tag="rhsm")
        rhsd = const.tile([128, 1], BF, tag="rhsd")
        ident = const.tile([128, 128], BF, tag="ident")
        nc.sync.dma_start(out=rhsf[:, :], in_=rhsf_d[:, :])
        nc.sync.dma_start(out=rhsb[:, :], in_=rhsb_d[:, :])
        nc.sync.dma_start(out=rhsm[:, :], in_=rhsm_d[:, :])
        nc.sync.dma_start(out=rhsd[64:128, :], in_=rhsd_d[:, :])
        nc.sync.dma_start(out=ident[:, :], in_=id_d[:, :])

        # persistent state
        c_all = state.tile([128, 96 * NT], BF, tag="c_all")  # c_b | c_f | c_m
        Fbs = [
            state.tile([32, BS], BF, tag=f"Fb{i}", name=f"Fb{i}") for i in range(2)
        ]
        Abs_ = [
            state.tile([32, BS], BF, tag=f"Ab{i}", name=f"Ab{i}") for i in range(2)
        ]
        V2s = [
            state.tile([128, BS], BF, tag=f"V2{i}", name=f"V2{i}") for i in range(2)
        ]
        out_sb = state.tile([128, t_steps * NT], F32, tag="out_sb")
        nc.sync.dma_start(out=c_all[:, :], in_=c0_d[:, :])

        C_B = slice(0, 16 * NT)
        C_F = slice(16 * NT, 32 * NT)
        C_M = slice(32 * NT, 96 * NT)

        # ------------- phase 1: fwd + bwd LSTMs interleaved -------------
        with tc.tile_pool(name="ps1", bufs=2, space="PSUM") as ps1, tc.tile_pool(
            name="ps1t", bufs=2, space="PSUM"
        ) as ps1t:

            def small_step(lhs_buf, nxt_buf, rhs_w, c_sl, dir_tag):
                """One H=16 LSTM step over 16 b-tiles as 2 independent
                batch-half chains; hT evacuated into nxt_buf[0:16]."""
                sig = work.tile([128, 48 * NT], BF, tag="sig" + dir_tag)
                tg = work.tile([128, 16 * NT], BF, tag="tg" + dir_tag)
                tc_t = work.tile([128, 16 * NT], BF, tag="tc" + dir_tag)
                h = work.tile([128, 16 * NT], BF, tag="h" + dir_tag)
                t1 = work.tile([128, 16 * NT], BF, tag="t1" + dir_tag)
                t2 = work.tile([128, 16 * NT], BF, tag="t2" + dir_tag)
                sigv = sig[:, :].rearrange("p (t c) -> p t c", c=48)
                tgv = tg[:, :].rearrange("p (t c) -> p t c", c=16)
                cv = c_all[:, c_sl].rearrange("p (t c) -> p t c", c=16)
                t1v = t1[:, :].rearrange("p (t c) -> p t c", c=16)
                t2v = t2[:, :].rearrange("p (t c) -> p t c", c=16)
                tcv = tc_t[:, :].rearrange("p (t c) -> p t c", c=16)
                hv = h[:, :].rearrange("p (t c) -> p t c", c=16)
                for g in range(2):
                    sl = slice(8 * g, 8 * (g + 1))
                    psf = ps1.tile([128, 512], F32, tag="psf" + dir_tag)
                    for j in range(8):
                        jt = g * 8 + j
                        nc.tensor.matmul(
                            psf[:, j * 64 : (j + 1) * 64],
                            lhs_buf[0:25, jt * 128 : (jt + 1) * 128],
                            rhs_w[:, :],
                            start=True,
                            stop=True,
                        )
                    psv = psf[:, :].rearrange("p (t c) -> p t c", c=64)
                    nc.scalar.activation(sigv[:, sl, :], psv[:, :, 0:48], AF.Sigmoid)
                    nc.scalar.activation(tgv[:, sl, :], psv[:, :, 48:64], AF.Tanh)
                    nc.vector.tensor_mul(
                        t1v[:, sl, :], sigv[:, sl, 16:32], cv[:, sl, :]
                    )
                    nc.gpsimd.tensor_mul(
                        t2v[:, sl, :], sigv[:, sl, 0:16], tgv[:, sl, :]
                    )
                    nc.vector.tensor_add(cv[:, sl, :], t1v[:, sl, :], t2v[:, sl, :])
                    nc.scalar.activation(tcv[:, sl, :], cv[:, sl, :], AF.Tanh)
                    nc.gpsimd.tensor_mul(
                        hv[:, sl, :], sigv[:, sl, 32:48], tcv[:, sl, :]
                    )
                    pst = ps1t.tile([16, 1024], BF, tag="pst" + dir_tag)
                    for j in range(8):
                        jt = g * 8 + j
                        nc.tensor.transpose(
                            pst[0:16, j * 128 : (j + 1) * 128],
                            h[:, jt * 16 : (jt + 1) * 16],
                            ident[:, :],
                        )
                    nc.vector.tensor_copy(
                        nxt_buf[0:16, g * 1024 : (g + 1) * 1024], pst[0:16, :]
                    )

            nc.sync.dma_start(out=Fbs[0][0:16, :], in_=h0f_d[:, :])
            nc.sync.dma_start(out=Fbs[0][16:32, :], in_=xpad_d[0])
            nc.sync.dma_start(out=Abs_[0][0:16, :], in_=h0b_d[:, :])
            nc.sync.dma_start(out=Abs_[0][16:32, :], in_=xpad_d[t_steps - 1])
            for k in range(t_steps):
                tf = k
                tb = t_steps - 1 - k
                cur, nxt = k % 2, (k + 1) % 2
                for (bufs, rhs_w, c_sl, tag, t_cur, t_next, row0) in (
                    (Fbs, rhsf, C_F, "f", tf, tf + 1, 0),
                    (Abs_, rhsb, C_B, "b", tb, tb - 1, 16),
                ):
                    if 0 <= t_next < t_steps:
                        nc.sync.dma_start(out=bufs[nxt][16:32, :], in_=xpad_d[t_next])
                    small_step(bufs[cur], bufs[nxt], rhs_w, c_sl, tag)
                    nc.sync.dma_start(
                        out=hfb_d[t_cur, row0 : row0 + 16, :], in_=bufs[nxt][0:16, :]
                    )

        tc.strict_bb_all_engine_barrier()

        # ------------- phase 2: mid LSTM + fused dense -------------
        with tc.tile_pool(name="ps2", bufs=1, space="PSUM") as ps2, tc.tile_pool(
            name="ps2t", bufs=2, space="PSUM"
        ) as ps2t:
            nc.sync.dma_start(out=V2s[0][0:32, :], in_=hfb_d[0, :, :])
            nc.sync.dma_start(out=V2s[0][32:64, :], in_=ones_d[:, :])
            nc.sync.dma_start(out=V2s[1][32:64, :], in_=ones_d[:, :])
            nc.sync.dma_start(out=V2s[0][64:128, :], in_=h0m_d[:, :])
            for t in range(t_steps):
                cur, nxt = t % 2, (t + 1) % 2
                V2, V2n = V2s[cur], V2s[nxt]
                if t < t_steps - 1:
                    nc.sync.dma_start(out=V2n[0:32, :], in_=hfb_d[t + 1, :, :])
                sigm = work.tile([128, 192 * NT], BF, tag="sigm")
                tgm = work.tile([128, 64 * NT], BF, tag="tgm")
                tcm = work.tile([128, 64 * NT], BF, tag="tcm")
                hm = work.tile([128, 64 * NT], BF, tag="hm")
                t1m = work.tile([128, 64 * NT], BF, tag="t1m")
                t2m = work.tile([128, 64 * NT], BF, tag="t2m")
                sigmv = sigm[:, :].rearrange("p (t c) -> p t c", c=192)
                tgmv = tgm[:, :].rearrange("p (t c) -> p t c", c=64)
                cmv = c_all[:, C_M].rearrange("p (t c) -> p t c", c=64)
                t1mv = t1m[:, :].rearrange("p (t c) -> p t c", c=64)
                t2mv = t2m[:, :].rearrange("p (t c) -> p t c", c=64)
                tcmv = tcm[:, :].rearrange("p (t c) -> p t c", c=64)
                hmv = hm[:, :].rearrange("p (t c) -> p t c", c=64)
                for g in range(4):
                    sl = slice(4 * g, 4 * (g + 1))
                    psm = ps2.tile([128, 2048], F32, tag="psm")
                    for k2 in range(4):
                        jt = 4 * g + k2
                        nc.tensor.matmul(
                            psm[:, k2 * 512 : k2 * 512 + 257],
                            V2[0:128, jt * 128 : (jt + 1) * 128],
                            rhsm[:, :],
                            start=True,
                            stop=True,
                        )
                    psv = psm[:, :].rearrange("p (t c) -> p t c", c=512)
                    nc.scalar.activation(sigmv[:, sl, :], psv[:, :, 0:192], AF.Sigmoid)
                    nc.scalar.activation(tgmv[:, sl, :], psv[:, :, 192:256], AF.Tanh)
                    if t >= 1:
                        nc.scalar.copy(
                            out_sb[
                                :, (t - 1) * 16 + 4 * g : (t - 1) * 16 + 4 * g + 4
                            ].rearrange("p (a b) -> p a b", b=1),
                            psv[:, :, 256:257],
                        )
                    nc.vector.tensor_mul(
                        t1mv[:, sl, :], sigmv[:, sl, 64:128], cmv[:, sl, :]
                    )
                    nc.gpsimd.tensor_mul(
                        t2mv[:, sl, :], sigmv[:, sl, 0:64], tgmv[:, sl, :]
                    )
                    nc.vector.tensor_add(cmv[:, sl, :], t1mv[:, sl, :], t2mv[:, sl, :])
                    nc.scalar.activation(tcmv[:, sl, :], cmv[:, sl, :], AF.Tanh)
                    nc.gpsimd.tensor_mul(
                        hmv[:, sl, :], sigmv[:, sl, 128:192], tcmv[:, sl, :]
                    )
                    pstm = ps2t.tile([64, 512], BF, tag="pstm")
                    for j in range(4):
                        jt = 4 * g + j
                        nc.tensor.transpose(
                            pstm[0:64, j * 128 : (j + 1) * 128],
                            hm[:, jt * 64 : (jt + 1) * 64],
                            ident[:, :],
                        )
                    nc.vector.tensor_copy(
                        V2n[64:128, g * 512 : (g + 1) * 512], pstm[0:64, :]
                    )

            # final dense tap: out[T-1] = Wd @ h_m[T-1] (+bd host-side)
            psd = ps2.tile([128, 2048], F32, tag="psm")
            Vlast = V2s[t_steps % 2]
            for j in range(NT):
                nc.tensor.matmul(
                    psd[:, j : j + 1],
                    Vlast[64:128, j * 128 : (j + 1) * 128],
                    rhsd[64:128, :],
                    start=True,
                    stop=True,
                )
            nc.vector.tensor_copy(
                out_sb[:, (t_steps - 1) * 16 : t_steps * 16], psd[:, 0:16]
            )
            nc.sync.dma_start(out=out_d[:, :], in_=out_sb[:, :])

    nc.finalize()
    return nc


def prepare_inputs(inputs, t_steps=T):
    """Build the per-core input maps (list of dicts) from full inputs."""
    f32 = np.float32
    x = np.asarray(inputs["x"], dtype=f32)[:t_steps]  # [T, B, 8]

    p1 = _perm4(H1)
    p2 = _perm4(H2)

    def rhs_small(Wih, Whh, bih, bhh):
        # rows 0:16 Whh.T ; 16:24 Wih.T ; 24 bias   (cols = gates (i,f,o,g))
        Wih = np.asarray(Wih, f32)[p1]
        Whh = np.asarray(Whh, f32)[p1]
        b = (np.asarray(bih, f32) + np.asarray(bhh, f32))[p1]
        out = np.zeros((25, 4 * H1), f32)
        out[0:16] = Whh.T
        out[16:24] = Wih.T
        out[24] = b
        return out.astype(BF16NP)

    rhs_f = rhs_small(
        inputs["Wih_f"], inputs["Whh_f"], inputs["bih_f"], inputs["bhh_f"]
    )
    rhs_b = rhs_small(
        inputs["Wih_b"], inputs["Whh_b"], inputs["bih_b"], inputs["bhh_b"]
    )

    Wih_m = np.asarray(inputs["Wih_m"], f32)[p2]  # [256, 32]
    Whh_m = np.asarray(inputs["Whh_m"], f32)[p2]  # [256, 64]
    b_m = (np.asarray(inputs["bih_m"], f32) + np.asarray(inputs["bhh_m"], f32))[p2]
    Wd = np.asarray(inputs["Wd"], f32)[0]  # [64]
    bd = np.asarray(inputs["bd"], f32)[0]
    rhs_m = np.zeros((128, 257), f32)
    rhs_m[0:16, 0:256] = Wih_m[:, 0:16].T  # h_f part (V2 rows 0:16)
    rhs_m[16:32, 0:256] = Wih_m[:, 16:32].T  # h_b part (V2 rows 16:32)
    rhs_m[32, 0:256] = b_m  # ones row (V2 row 32) -> bias
    rhs_m[64:128, 0:256] = Whh_m.T  # h_m part (V2 rows 64:128)
    rhs_m[32, 256] = bd
    rhs_m[64:128, 256] = Wd
    rhs_m = rhs_m.astype(BF16NP)

    rhs_d = Wd.reshape(64, 1).astype(BF16NP)  # bd added host-side for last col

    ident = np.eye(128, dtype=BF16NP)
    onespad = np.zeros((32, BS), BF16NP)
    onespad[0, :] = 1

    in_maps = []
    for c in range(NCORES):
        bs, be = c * BS, (c + 1) * BS
        xc = x[:, bs:be, :]  # [T, 2048, 8]
        xpad = np.zeros((t_steps, 16, BS), BF16NP)
        xpad[:, 0:8, :] = xc.transpose(0, 2, 1).astype(BF16NP)
        xpad[:, 8, :] = np.ones((BS,), BF16NP)

        def bm(a, H):  # [BS, H] -> batch-major [128, NT*H]
            return (
                np.asarray(a, f32)[bs:be]
                .reshape(NT, 128, H)
                .transpose(1, 0, 2)
                .reshape(128, NT * H)
            )

        c0 = np.zeros((128, 96 * NT), f32)
        c0[:, 0 : 16 * NT] = bm(inputs["c0b"], H1)
        c0[:, 16 * NT : 32 * NT] = bm(inputs["c0f"], H1)
        c0[:, 32 * NT :] = bm(inputs["c0m"], H2)

        in_maps.append(
            {
                "xpad": xpad,
                "rhs_f": rhs_f,
                "rhs_b": rhs_b,
                "rhs_m": rhs_m,
                "rhs_d": rhs_d,
                "h0fT": np.asarray(inputs["h0f"], f32)[bs:be].T.astype(BF16NP),
                "h0bT": np.asarray(inputs["h0b"], f32)[bs:be].T.astype(BF16NP),
                "h0mT": np.asarray(inputs["h0m"], f32)[bs:be].T.astype(BF16NP),
                "c0": c0.astype(BF16NP),
                "ident": ident,
                "onespad": onespad,
            }
        )
    return in_maps


def unshard_output(results, bd, t_steps=T):
    outs = []
    for c in range(NCORES):
        oc = np.asarray(results[c]["out"], dtype=np.float32)  # [128, T*NT]
        # col = t*NT + b-tile index
        oc = oc.reshape(128, t_steps, NT).transpose(2, 0, 1).reshape(BS, t_steps)
        outs.append(oc)
    full = np.concatenate(outs, axis=0)  # [B, T]
    full[:, t_steps - 1] += bd  # last step's dense bias is added host-side
    return full


_CACHED = {}


def kernel(**inputs):
    from concourse.bass_utils import run_bass_kernel_spmd

    t_steps = T
    if "prog" not in _CACHED:
        _CACHED["prog"] = build_program(t_steps)
    nc = _CACHED["prog"]
    in_maps = prepare_inputs(inputs, t_steps)
    res = run_bass_kernel_spmd(nc, in_maps, list(range(NCORES)))
    bd = float(np.asarray(inputs["bd"], np.float32)[0])
    return unshard_output(res.results, bd, t_steps)


if __name__ == "__main__":
    import reference

    inputs = reference.setup_inputs()
    out = kernel(**{k: np.asarray(v) for k, v in inputs.items()})
    print("kernel out", out.shape, out.dtype)

